# revision 10
# baseline (speedup 1.0000x reference)
"""Trainium2 Bass kernel for LogicalConsistencyLoss.

loss = W/(R*B) * sum_{b,r} sum_{a,i,c} relu(rel[a,i] - rel[a,c]*rel[i,c])
with rel = sigmoid(logits[b,:,:,r]) masked by the entity_masks outer product
(host folds the mask into the logits as -30).

Distribution: B*R = 8 (batch, relation) matrices -> 8 NeuronCores, one
512x512 matrix per core. Each core returns its scalar partial sum; the
host combines.

Per-core split of the N^3 relu work, by c (the contracted transitivity
index). For each fixed c the violation matrix is
    viol[b_, a] = relu(rel[a, b_] - rel[a, c] * rel[b_, c])
  - c in [0, CA) (pipeline A, DVE): layout partition=b_, free=a.
    Src1 = relT (bf16, SBUF), Src0 = row c of relT broadcast across all
    128 partitions (bf16, SBUF, produced by a DMA broadcast straight from
    a DRAM copy of relT), C0 = rel[:, c] per-partition scalar. One fused
    custom DVE op relu(Src1 - Src0*C0) with free-dim sum into
    acc_a[:, c]. All non-scalar operands are packed bf16 in SBUF, and the
    op declares the full DVE perf-mode ladder (2x_1p/2x_2p/4x_2p table
    programs + perf_max=3), so the op runs in 4x_2p mode: 0.25
    elem/cycle-lane fp32-equivalent -> ~194 ns per [128,512] tile.
  - c in [CA, 512) (pipeline B, PE+ACT): layout partition=a, free=b_.
    Per (c, a-tile): PE writes -rel into PSUM ((-I)^T @ relb) then
    accumulates +col_c (x) col_c (K=1 matmul; rows live in a flat
    base-partition-aligned row store filled via the DRAM copy of relT);
    ScalarE applies Relu(scale=-1) with accum_out over a 4-bank
    [128,2048] PSUM tile (four c's of one a-tile per activation).
The c split CA is chosen so DVE and ACT finish together; PE and the DMA
ring both run below both.
"""

import sys

if "/opt/trn_rl_repo" not in sys.path:
    sys.path.insert(0, "/opt/trn_rl_repo")

import numpy as np
import ml_dtypes

N = 512
P = 128
NT = N // P          # 4 row tiles
# DVE perf mode for the custom op: 0 = 1x only (validated on HW);
# 1 = 2x_1p (hand-built table program; accumulator path not yet working
# on silicon). CA rebalances the c split so DVE and ACT finish together.
PERF_MAX = 0
CA = 236 if PERF_MAX == 0 else 312  # c < CA -> pipeline A (DVE)
ABATCH = 4           # c's per broadcast DMA batch
NB_ROWS = N - CA               # rows in the flat store
NBG = NB_ROWS // 4             # B c-groups of 4
GROUP = (NB_ROWS + 2) // 3     # rows per base-partition group
TEMPERATURE = 1.0
WEIGHT = 1.0

_CACHE: dict = {}


def _flat_loc(j):
    """Row j (= c - CA) of the flat store -> (base_partition, elem_offset)."""
    g, q = j // GROUP, j % GROUP
    return 32 * g, q * N


def _get_custom_op():
    """Register (once) the fused DVE op: out = relu(Src1 - Src0*C0),
    accum_out = sum(out). The op ships table programs for every DVE perf
    mode (the elementwise body is mode-agnostic) and is emitted with
    perf_max=3 so the engine may run it in 2x/4x mode when operand
    dtype/stride/space allow."""
    import concourse.dve_ops as dve_ops
    from concourse.dve_spec import Spec, Src0, Src1, C0, relu, lower
    from concourse.dve_uop import DveOpSpec
    from concourse.dve_table_gen import dve_ver_for

    name = "LCL_RSUB_MUL_RELU_SUM"
    for o in dve_ops.OPS:
        if o.name == name:
            return o

    def _ref(in0, in1, s0, s1, imm2):
        out = np.maximum(np.asarray(in1, np.float32)
                         - np.asarray(in0, np.float32) * s0, 0.0)
        return out, out.reshape(out.shape[0], -1).sum(axis=1, keepdims=True)

    from operator import add

    spec = Spec(body=relu(Src1 - Src0 * C0), accum=add, reference=_ref)
    opc = max(dve_ops._SUB_OPCODE_FOR_NAME.values()) + 1
    assert opc < 0x20
    ver = dve_ver_for("TRN2")
    uops = lower(spec, ver=ver)
    if PERF_MAX > 0:
        from probe2x import build_2x_uops
        full = DveOpSpec(name=name, opcode=opc, uops=uops,
                         uops_2x=build_2x_uops(uops), perf_max=PERF_MAX,
                         rd1_en=True)
        full.validate(ver)
    else:
        full = DveOpSpec(name=name, opcode=opc, uops=uops, rd1_en=True)
    sha = full.sha(ver)
    op = dve_ops.DveOp(name, spec, subdim=False, uops_sha={ver: sha})
    dve_ops._SUB_OPCODE_FOR_NAME[name] = opc
    dve_ops.OPS.append(op)
    dve_ops._COMPILE_CACHE[(name, ver)] = full
    return op


def _build():
    import concourse.bacc as bacc
    import concourse.mybir as mybir
    from concourse.tile import TileContext

    f32 = mybir.dt.float32
    bf16 = mybir.dt.bfloat16
    OP = _get_custom_op()

    nc = bacc.Bacc("TRN2", target_bir_lowering=False)
    x = nc.dram_tensor("x", [N, N], f32, kind="ExternalInput")
    ident32_d = nc.dram_tensor("ident32", [P, P], f32, kind="ExternalInput")
    identbn_d = nc.dram_tensor("identbn", [P, P], bf16, kind="ExternalInput")
    ones32_d = nc.dram_tensor("ones32", [P, 1], f32, kind="ExternalInput")
    out_d = nc.dram_tensor("out", [1, 1], f32, kind="ExternalOutput")

    with TileContext(nc) as tc:
        with (
            tc.tile_pool(name="const", bufs=1) as cp,
            tc.tile_pool(name="brow", bufs=3) as bp,
            tc.tile_pool(name="scr_a", bufs=3) as sa,
            tc.tile_pool(name="dram", bufs=1, space="DRAM") as dp,
            tc.tile_pool(name="pb", bufs=2, space="PSUM") as pbp,
        ):
            ident32 = cp.tile([P, P], f32, tag="ident32", name="ident32")
            identbn = cp.tile([P, P], bf16, tag="identbn", name="identbn")
            ones32 = cp.tile([P, 1], f32, tag="ones32", name="ones32")
            nc.sync.dma_start(out=ident32, in_=ident32_d[:, :])
            nc.sync.dma_start(out=identbn, in_=identbn_d[:, :])
            nc.sync.dma_start(out=ones32, in_=ones32_d[:, :])

            xt = [cp.tile([P, N], f32, tag=f"xt{t}", name=f"xt{t}")
                  for t in range(NT)]
            rel32 = [cp.tile([P, N], f32, tag=f"rel32{t}", name=f"rel32{t}")
                     for t in range(NT)]
            relb = [cp.tile([P, N], bf16, tag=f"relb{t}", name=f"relb{t}")
                    for t in range(NT)]
            relTb = [cp.tile([P, N], bf16, tag=f"relTb{t}", name=f"relTb{t}")
                     for t in range(NT)]
            flat = cp.tile([P, GROUP * N], bf16, tag="flat", name="flat")
            acc_a = [cp.tile([P, CA], f32, tag=f"acca{t}", name=f"acca{t}")
                     for t in range(NT)]
            acc_b = [cp.tile([P, NBG], f32, tag=f"accb{t}", name=f"accb{t}")
                     for t in range(NT)]

            for t in range(NT):
                nc.sync.dma_start(out=xt[t], in_=x[t * P:(t + 1) * P, :])
            for t in range(NT):
                nc.scalar.activation(
                    rel32[t], xt[t], mybir.ActivationFunctionType.Sigmoid,
                    scale=1.0 / TEMPERATURE,
                )
                nc.vector.tensor_copy(relb[t], rel32[t])
            # transpose rel32 -> relTb (16 PE 128x128 blocks, one PSUM->SBUF
            # bf16 copy per destination tile)
            for tcol in range(NT):
                pt = pbp.tile([P, 4 * N], f32, tag="pb", name="pb")
                for t in range(NT):
                    nc.tensor.transpose(
                        pt[:, t * P:(t + 1) * P],
                        rel32[t][:, tcol * P:(tcol + 1) * P], ident32,
                    )
                nc.vector.tensor_copy(relTb[tcol], pt[:, :N])
            # full relT lives in DRAM: broadcast source for pipeline A and
            # flat-store source for pipeline B
            relT_dram = dp.tile([N, N], bf16, name="relT_dram")
            for t in range(NT):
                nc.sync.dma_start(
                    out=relT_dram[t * P:(t + 1) * P, :], in_=relTb[t]
                )
            # flat row store: relT rows c in [CA, 512) at base partitions
            # {0,32,64}
            for g in range(3):
                r0 = g * GROUP
                nrows = min(GROUP, NB_ROWS - r0)
                nc.sync.dma_start(
                    out=flat[32 * g:32 * g + 1, 0:nrows * N],
                    in_=relT_dram[CA + r0:CA + r0 + nrows, :],
                )

            # ---- main: interleave A batches (93) and B units (140) ----
            def emit_a_batch(bi):
                c0 = bi * ABATCH
                brow = bp.tile([P, ABATCH * N], bf16, tag="brow", name="brow")
                nc.sync.dma_start(
                    out=brow,
                    in_=relT_dram[c0:c0 + ABATCH, :].partition_broadcast(P),
                )
                for k in range(ABATCH):
                    c = c0 + k
                    for tb in range(NT):
                        so = sa.tile([P, N], bf16, tag="scr_a", name="scr_a")
                        nc.vector._custom_dve(
                            OP,
                            out=so,
                            in0=brow[:, k * N:(k + 1) * N],
                            in1=relTb[tb],
                            s0=rel32[tb][:, c:c + 1],
                            accum_out=acc_a[tb][:, c:c + 1],
                        )

            def emit_b_unit(ui):
                g, tb = ui // NT, ui % NT
                pbt = pbp.tile([P, 4 * N], f32, tag="pb", name="pb")
                for k in range(4):
                    c = CA + 4 * g + k
                    bpar, off = _flat_loc(c - CA)
                    half = pbt[:, k * N:(k + 1) * N]
                    nc.tensor.matmul(half, identbn, relb[tb],
                                     start=True, stop=False)
                    nc.tensor.matmul(
                        half,
                        flat[bpar:bpar + 1, off + tb * P:off + (tb + 1) * P],
                        flat[bpar:bpar + 1, off:off + N],
                        start=False, stop=True,
                    )
                nc.scalar.activation(
                    pbt, pbt, mybir.ActivationFunctionType.Relu,
                    scale=-1.0,
                    accum_out=acc_b[tb][:, g:g + 1],
                )

            n_ab = CA // ABATCH          # 93
            n_bu = NBG * NT              # 140
            a_emitted = 0
            for ui in range(n_bu):
                while a_emitted * n_bu < n_ab * (ui + 1):
                    emit_a_batch(a_emitted)
                    a_emitted += 1
                emit_b_unit(ui)
            while a_emitted < n_ab:
                emit_a_batch(a_emitted)
                a_emitted += 1

            # ---- final reduction ----
            parts = []
            for t in range(NT):
                r = cp.tile([P, 1], f32, tag=f"ra{t}", name=f"ra{t}")
                nc.vector.tensor_reduce(
                    r, acc_a[t], axis=mybir.AxisListType.X, op=mybir.AluOpType.add
                )
                parts.append(r)
            for t in range(NT):
                r = cp.tile([P, 1], f32, tag=f"rb{t}", name=f"rb{t}")
                nc.vector.tensor_reduce(
                    r, acc_b[t], axis=mybir.AxisListType.X, op=mybir.AluOpType.add
                )
                parts.append(r)
            tot = parts[0]
            for r in parts[1:]:
                nc.vector.tensor_add(tot, tot, r)
            pt = pbp.tile([P, 4 * N], f32, tag="pb", name="pb")
            nc.tensor.matmul(pt[0:1, 0:1], tot, ones32, start=True, stop=True)
            out_sb = cp.tile([1, 1], f32, tag="out_sb", name="out_sb")
            nc.vector.tensor_copy(out_sb, pt[0:1, 0:1])
            nc.sync.dma_start(out=out_d[:, :], in_=out_sb)

    # Declare the full perf-mode ladder on every emitted custom-DVE
    # instruction (byte-36[7:6] of the encoding; must precede compile so
    # the NEFF carries it). The op's table ships programs for all four
    # mode slots, so the engine may legally engage 2x/4x.
    import concourse.mybir as mybir
    fn = nc.m.functions[0]
    for bb in fn.blocks:
        for inst in bb.instructions:
            if isinstance(inst, mybir.InstCustomDveAnt):
                inst.perf_max = PERF_MAX

    nc.compile()
    return nc


def _get_nc():
    if "nc" not in _CACHE:
        _CACHE["nc"] = _build()
    return _CACHE["nc"]


def _consts():
    if "consts" not in _CACHE:
        _CACHE["consts"] = {
            "ident32": np.eye(P, dtype=np.float32),
            "identbn": (-np.eye(P)).astype(ml_dtypes.bfloat16),
            "ones32": np.ones((P, 1), dtype=np.float32),
        }
    return _CACHE["consts"]


def kernel(relation_logits: np.ndarray, entity_masks: np.ndarray) -> np.ndarray:
    from concourse.bass_utils import run_bass_kernel_spmd

    B, n, _, R = relation_logits.shape
    assert (n, B * R) == (N, 8)
    x = np.ascontiguousarray(
        np.transpose(np.asarray(relation_logits, dtype=np.float32), (0, 3, 1, 2))
    ).reshape(B * R, N, N)
    m = np.asarray(entity_masks) > 0
    for b in range(B):
        if not m[b].all():
            keep = np.outer(m[b], m[b])
            x[b * R:(b + 1) * R][:, ~keep] = -30.0

    consts = _consts()
    in_maps = [{"x": x[i], **consts} for i in range(8)]
    res = run_bass_kernel_spmd(_get_nc(), in_maps, list(range(8)))
    total = float(sum(float(r["out"][0, 0]) for r in res.results))
    return np.float32(WEIGHT * total / (R * B))


# revision 17
# speedup vs baseline: 1.0799x; 1.0799x over previous
"""Trainium2 Bass kernel for LogicalConsistencyLoss.

loss = W/(R*B) * sum_{b,r} sum_{a,i,c} relu(rel[a,i] - rel[a,c]*rel[i,c])
with rel = sigmoid(logits[b,:,:,r]) masked by the entity_masks outer product
(host folds the mask into the logits as -30).

Distribution: B*R = 8 (batch, relation) matrices -> 8 NeuronCores, one
512x512 matrix per core. Each core returns its scalar partial sum; the
host combines.

Per-core split of the N^3 relu work, by c (the contracted transitivity
index). For each fixed c the violation matrix (partition=b_, free=a for
the DVE pipelines) is  viol[b_, a] = relu(rel[a,b_] - rel[a,c]*rel[b_,c])
 = relu(Src1 - Src0*C0) with Src1 = relT tile (bf16 SBUF), Src0 = row c
of relT broadcast across partitions (bf16 SBUF, DMA-broadcast straight
from a DRAM copy of relT), C0 = rel[:, c] per-partition scalar.

Three parallel consumers of the c range:
  - c in [0, C_FUSED): DVE 1x fused custom op (relu + free-dim sum into
    acc_a[:, c]). The DVE accumulator only works in regular mode on this
    silicon, so these run at 1 elem/cycle-lane.
  - c in [C_FUSED, CA): DVE 2x_1p custom op WITHOUT accumulator (the
    hand-built dual-stream table program; elementwise path verified on
    HW) writes relu tiles to scratch; the otherwise-idle GPSIMD engine
    accumulates them elementwise into per-b-tile accT tiles.
  - c in [CA, 512) (PE+ACT): per (c, a-tile) PE writes -rel into PSUM
    ((-I)^T @ relb) then adds +col_c (x) col_c (K=1 matmul from a flat
    base-partition-aligned row store); ScalarE applies Relu(scale=-1)
    with accum_out over a 4-bank [128,2048] PSUM tile (4 c's per
    activation).
Split sizes balance DVE / ACT / GPSIMD; PE and the DMA broadcast ring
run below all three.
"""

import sys

if "/opt/trn_rl_repo" not in sys.path:
    sys.path.insert(0, "/opt/trn_rl_repo")

import copy

import numpy as np
import ml_dtypes

N = 512
P = 128
NT = N // P          # 4 row tiles
C_FUSED = 152        # c's on the DVE 1x fused op
C_POOL = 112         # c's on the DVE 2x no-accum op + GPSIMD accumulate
CA = C_FUSED + C_POOL          # start of the PE+ACT range
ABATCH = 4           # c's per broadcast DMA batch
NB_ROWS = N - CA               # rows in the flat store
NBG = NB_ROWS // 4             # B c-groups of 4
GROUP = (NB_ROWS + 2) // 3     # rows per base-partition group
TEMPERATURE = 1.0
WEIGHT = 1.0

_CACHE: dict = {}


def _flat_loc(j):
    """Row j (= c - CA) of the flat store -> (base_partition, elem_offset)."""
    g, q = j // GROUP, j % GROUP
    return 32 * g, q * N


def _build_2x_uops(uops_1x):
    """Hand-assembled 2X_1PORT table program for relu(Src1 - Src0*C0).
    Each cycle consumes an even/odd element pair: even chain at stages
    0-2, odd chain at 3-5 (odd inputs ride the delay lines), outputs
    written through the lo/hi write ports from stage-7 delay taps. The
    elementwise path is HW-verified; the stream accumulator does NOT
    survive fast mode, so instructions using this mode must not take
    accum_out."""
    from concourse.dve_uop import InpSel, OutSel, OutPath, AluInp, DelayInp, AluOp

    PD, PA = DelayInp.PREV_DELAY, DelayInp.PREV_ALU_OUT
    A = AluInp
    u0, u1 = copy.deepcopy(uops_1x[0]), copy.deepcopy(uops_1x[1])
    # lanes: 0=Src1_odd (read via PREV_ALU_OUT at stage0), 1=Src1_even,
    # 2=Src0_even, 3=C0, 4=zero, 5=Src0_odd
    lanes = [InpSel.SRC_1_HI, InpSel.SRC_1, InpSel.SRC_0, InpSel.CONST_0,
             InpSel.ZERO, InpSel.SRC_0_HI, InpSel.ZERO, InpSel.ZERO]
    en = [1, 1, 1, 1, 1, 1, 0, 0]
    for u in (u0, u1):
        u.inp = list(lanes)
        u.inp_enable = list(en)

    M, SU, MX, AD, BY = (AluOp.MULTIPLY, AluOp.SUBTRACT, AluOp.MAX,
                         AluOp.ADD, AluOp.BYPASS)

    def cfg(dp, s, op, a, b, delay, den, accA=0):
        d = dp[s]
        d.op = op
        d.alu_src0 = a
        d.alu_src1 = b
        d.delay = list(delay)
        d.delay_enable = list(den)
        d.alu_out_a_enable = accA
        d.alu_out_b_enable = 0
        d.alu_out_enable = 1

    dp = u1.datapath_config
    # stage0: m_e = Src0_e*C0; d1 captures Src1_odd (lane0 via PA)
    cfg(dp, 0, M, A.PREV_DELAY_1, A.PREV_DELAY_2,
        [PD, PA, PD, PD, PD, PD, PD], [1, 1, 1, 1, 1, 0, 0])
    # stage1: s_e = Src1_e - m_e
    cfg(dp, 1, SU, A.PREV_DELAY_0, A.PREV_ALU_OUT,
        [PD, PD, PD, PD, PD, PD, PD], [1, 1, 1, 1, 1, 0, 0])
    # stage2: r_e = max(s_e, 0)
    cfg(dp, 2, MX, A.PREV_ALU_OUT, A.PREV_DELAY_3,
        [PD, PD, PD, PD, PD, PD, PD], [1, 1, 1, 1, 1, 0, 0])
    # stage3: m_o = Src0_o*C0; d0 captures r_e
    cfg(dp, 3, M, A.PREV_DELAY_4, A.PREV_DELAY_2,
        [PA, PD, PD, PD, PD, PD, PD], [1, 1, 1, 1, 1, 0, 0])
    # stage4: s_o = Src1_o - m_o
    cfg(dp, 4, SU, A.PREV_DELAY_1, A.PREV_ALU_OUT,
        [PD, PD, PD, PD, PD, PD, PD], [1, 1, 1, 1, 1, 0, 0])
    # stage5: r_o = max(s_o, 0)
    cfg(dp, 5, MX, A.PREV_ALU_OUT, A.PREV_DELAY_3,
        [PD, PD, PD, PD, PD, PD, PD], [1, 1, 1, 1, 1, 0, 0])
    # stage6: d1 captures r_o; d3 keeps the zero flowing for the init uop
    cfg(dp, 6, AD, A.PREV_DELAY_0, A.PREV_ALU_OUT,
        [PD, PA, PD, PD, PD, PD, PD], [1, 1, 0, 1, 0, 0, 0])
    # stage7: d0/d1 carry r_e/r_o to the writes
    cfg(dp, 7, AD, A.CURR_ALU_OUT, A.PREV_ALU_OUT,
        [PD, PD, PD, PD, PD, PD, PD], [1, 1, 0, 1, 0, 0, 0], accA=1)
    u1.out = {OutPath.WR0_LO: OutSel.DELAY_0, OutPath.WR0_HI: OutSel.DELAY_1,
              OutPath.WR1_LO: OutSel.ALU_OUT, OutPath.WR1_HI: OutSel.ALU_OUT}
    u1.out_enable = {OutPath.WR0_LO: 1, OutPath.WR0_HI: 1,
                     OutPath.WR1_LO: 0, OutPath.WR1_HI: 0}

    # init uop: one cycle, no consume; zero the accumulator register
    dp0 = u0.datapath_config
    for s in range(8):
        d1 = dp[s]
        d0 = dp0[s]
        d0.op = d1.op
        d0.alu_src0 = d1.alu_src0
        d0.alu_src1 = d1.alu_src1
        d0.delay = list(d1.delay)
        d0.delay_enable = list(d1.delay_enable)
        d0.alu_out_a_enable = d1.alu_out_a_enable
        d0.alu_out_enable = 1
    dp0[7].op = BY
    dp0[7].alu_src0 = A.PREV_DELAY_3
    dp0[7].alu_src1 = A.PREV_DELAY_3
    return [u0, u1]


def _get_custom_op():
    """Register (once) the fused DVE op: out = relu(Src1 - Src0*C0),
    accum_out = sum(out). The table also ships the hand-built 2X_1PORT
    program; instructions opt in per-call via perf_max."""
    import concourse.dve_ops as dve_ops
    from concourse.dve_spec import Spec, Src0, Src1, C0, relu, lower
    from concourse.dve_uop import DveOpSpec
    from concourse.dve_table_gen import dve_ver_for
    from operator import add

    name = "LCL_RSUB_MUL_RELU_SUM"
    for o in dve_ops.OPS:
        if o.name == name:
            return o

    def _ref(in0, in1, s0, s1, imm2):
        out = np.maximum(np.asarray(in1, np.float32)
                         - np.asarray(in0, np.float32) * s0, 0.0)
        return out, out.reshape(out.shape[0], -1).sum(axis=1, keepdims=True)

    spec = Spec(body=relu(Src1 - Src0 * C0), accum=add, reference=_ref)
    opc = max(dve_ops._SUB_OPCODE_FOR_NAME.values()) + 1
    assert opc < 0x20
    ver = dve_ver_for("TRN2")
    uops = lower(spec, ver=ver)
    full = DveOpSpec(name=name, opcode=opc, uops=uops,
                     uops_2x=_build_2x_uops(uops), rd1_en=True)
    full.validate(ver)
    sha = full.sha(ver)
    op = dve_ops.DveOp(name, spec, subdim=False, uops_sha={ver: sha})
    dve_ops._SUB_OPCODE_FOR_NAME[name] = opc
    dve_ops.OPS.append(op)
    dve_ops._COMPILE_CACHE[(name, ver)] = full
    return op


def _build():
    import concourse.bacc as bacc
    import concourse.mybir as mybir
    from concourse.tile import TileContext

    f32 = mybir.dt.float32
    bf16 = mybir.dt.bfloat16
    OP = _get_custom_op()

    nc = bacc.Bacc("TRN2", target_bir_lowering=False)
    x = nc.dram_tensor("x", [N, N], f32, kind="ExternalInput")
    ident32_d = nc.dram_tensor("ident32", [P, P], f32, kind="ExternalInput")
    identbn_d = nc.dram_tensor("identbn", [P, P], bf16, kind="ExternalInput")
    ones32_d = nc.dram_tensor("ones32", [P, 1], f32, kind="ExternalInput")
    out_d = nc.dram_tensor("out", [1, 1], f32, kind="ExternalOutput")

    noacc_insts = []

    with TileContext(nc) as tc:
        with (
            tc.tile_pool(name="const", bufs=1) as cp,
            tc.tile_pool(name="brow", bufs=4) as bp,
            tc.tile_pool(name="scr_a", bufs=3) as sa,
            tc.tile_pool(name="scr_p", bufs=32) as sp,
            tc.tile_pool(name="dram", bufs=1, space="DRAM") as dp,
            tc.tile_pool(name="pb", bufs=2, space="PSUM") as pbp,
        ):
            ident32 = cp.tile([P, P], f32, tag="ident32", name="ident32")
            identbn = cp.tile([P, P], bf16, tag="identbn", name="identbn")
            ones32 = cp.tile([P, 1], f32, tag="ones32", name="ones32")
            nc.sync.dma_start(out=ident32, in_=ident32_d[:, :])
            nc.sync.dma_start(out=identbn, in_=identbn_d[:, :])
            nc.sync.dma_start(out=ones32, in_=ones32_d[:, :])

            xt = [cp.tile([P, N], f32, tag=f"xt{t}", name=f"xt{t}")
                  for t in range(NT)]
            rel32 = [cp.tile([P, N], f32, tag=f"rel32{t}", name=f"rel32{t}")
                     for t in range(NT)]
            relb = [cp.tile([P, N], bf16, tag=f"relb{t}", name=f"relb{t}")
                    for t in range(NT)]
            relTb = [cp.tile([P, N], bf16, tag=f"relTb{t}", name=f"relTb{t}")
                     for t in range(NT)]
            flat = cp.tile([P, GROUP * N], bf16, tag="flat", name="flat")
            acc_a = [cp.tile([P, C_FUSED], f32, tag=f"acca{t}", name=f"acca{t}")
                     for t in range(NT)]
            acc_b = [cp.tile([P, NBG], f32, tag=f"accb{t}", name=f"accb{t}")
                     for t in range(NT)]
            accT = [cp.tile([P, N], f32, tag=f"accT{t}", name=f"accT{t}")
                    for t in range(NT)]

            for t in range(NT):
                nc.gpsimd.memset(accT[t], 0.0)
            for t in range(NT):
                nc.sync.dma_start(out=xt[t], in_=x[t * P:(t + 1) * P, :])
            for t in range(NT):
                nc.scalar.activation(
                    rel32[t], xt[t], mybir.ActivationFunctionType.Sigmoid,
                    scale=1.0 / TEMPERATURE,
                )
                nc.gpsimd.tensor_copy(relb[t], rel32[t])
            # transpose rel32 -> relTb (16 PE 128x128 blocks, one PSUM->SBUF
            # bf16 copy per destination tile; the copy runs on ScalarE —
            # GPSIMD cannot read PSUM and DVE is the bottleneck engine)
            for tcol in range(NT):
                pt = pbp.tile([P, 4 * N], f32, tag="pb", name="pb")
                for t in range(NT):
                    nc.tensor.transpose(
                        pt[:, t * P:(t + 1) * P],
                        rel32[t][:, tcol * P:(tcol + 1) * P], ident32,
                    )
                nc.scalar.activation(
                    relTb[tcol], pt[:, :N],
                    mybir.ActivationFunctionType.Copy,
                )
            # full relT in DRAM: broadcast source for the DVE pipelines and
            # flat-store source for the PE+ACT pipeline
            relT_dram = dp.tile([N, N], bf16, name="relT_dram")
            for t in range(NT):
                nc.sync.dma_start(
                    out=relT_dram[t * P:(t + 1) * P, :], in_=relTb[t]
                )
            for g in range(3):
                r0 = g * GROUP
                nrows = min(GROUP, NB_ROWS - r0)
                nc.sync.dma_start(
                    out=flat[32 * g:32 * g + 1, 0:nrows * N],
                    in_=relT_dram[CA + r0:CA + r0 + nrows, :],
                )

            # ---- main loops ----
            # Spread the pooled batches uniformly among the fused ones so
            # GPSIMD starts working immediately (it is the slowest per-unit
            # consumer; bunching its work at the end adds a serial tail).
            n_pb = C_POOL // ABATCH
            n_ab = CA // ABATCH
            pooled_batches = {
                bi for bi in range(n_ab)
                if (bi + 1) * n_pb // n_ab > bi * n_pb // n_ab
            }
            fused_col = [0]

            def emit_a_batch(bi):
                c0 = bi * ABATCH
                pooled = bi in pooled_batches
                brow = bp.tile([P, ABATCH * N], bf16, tag="brow", name="brow")
                nc.sync.dma_start(
                    out=brow,
                    in_=relT_dram[c0:c0 + ABATCH, :].partition_broadcast(P),
                )
                for k in range(ABATCH):
                    c = c0 + k
                    for tb in range(NT):
                        if pooled:
                            so = sp.tile([P, N], bf16, tag="scr_p",
                                         name="scr_p")
                            inst = nc.vector._custom_dve(
                                OP, out=so,
                                in0=brow[:, k * N:(k + 1) * N],
                                in1=relTb[tb],
                                s0=rel32[tb][:, c:c + 1],
                            )
                            noacc_insts.append(inst)
                            nc.gpsimd.tensor_add(accT[tb], accT[tb], so)
                        else:
                            so = sa.tile([P, N], bf16, tag="scr_a",
                                         name="scr_a")
                            nc.vector._custom_dve(
                                OP, out=so,
                                in0=brow[:, k * N:(k + 1) * N],
                                in1=relTb[tb],
                                s0=rel32[tb][:, c:c + 1],
                                accum_out=acc_a[tb][:, fused_col[0]:
                                                    fused_col[0] + 1],
                            )
                    if not pooled:
                        fused_col[0] += 1

            def emit_b_unit(ui):
                g, tb = ui // NT, ui % NT
                pbt = pbp.tile([P, 4 * N], f32, tag="pb", name="pb")
                for k in range(4):
                    c = CA + 4 * g + k
                    bpar, off = _flat_loc(c - CA)
                    half = pbt[:, k * N:(k + 1) * N]
                    nc.tensor.matmul(half, identbn, relb[tb],
                                     start=True, stop=False)
                    nc.tensor.matmul(
                        half,
                        flat[bpar:bpar + 1, off + tb * P:off + (tb + 1) * P],
                        flat[bpar:bpar + 1, off:off + N],
                        start=False, stop=True,
                    )
                nc.scalar.activation(
                    pbt, pbt, mybir.ActivationFunctionType.Relu,
                    scale=-1.0,
                    accum_out=acc_b[tb][:, g:g + 1],
                )

            n_bu = NBG * NT
            a_emitted = 0
            for ui in range(n_bu):
                while a_emitted * n_bu < n_ab * (ui + 1):
                    emit_a_batch(a_emitted)
                    a_emitted += 1
                emit_b_unit(ui)
            while a_emitted < n_ab:
                emit_a_batch(a_emitted)
                a_emitted += 1

            # ---- final reduction ----
            parts = []
            for t in range(NT):
                r = cp.tile([P, 1], f32, tag=f"ra{t}", name=f"ra{t}")
                nc.vector.tensor_reduce(
                    r, acc_a[t], axis=mybir.AxisListType.X, op=mybir.AluOpType.add
                )
                parts.append(r)
            for t in range(NT):
                r = cp.tile([P, 1], f32, tag=f"rb{t}", name=f"rb{t}")
                nc.vector.tensor_reduce(
                    r, acc_b[t], axis=mybir.AxisListType.X, op=mybir.AluOpType.add
                )
                parts.append(r)
            for t in range(NT):
                r = cp.tile([P, 1], f32, tag=f"rt{t}", name=f"rt{t}")
                nc.vector.tensor_reduce(
                    r, accT[t], axis=mybir.AxisListType.X, op=mybir.AluOpType.add
                )
                parts.append(r)
            tot = parts[0]
            for r in parts[1:]:
                nc.vector.tensor_add(tot, tot, r)
            pt = pbp.tile([P, 4 * N], f32, tag="pb", name="pb")
            nc.tensor.matmul(pt[0:1, 0:1], tot, ones32, start=True, stop=True)
            out_sb = cp.tile([1, 1], f32, tag="out_sb", name="out_sb")
            nc.vector.tensor_copy(out_sb, pt[0:1, 0:1])
            nc.sync.dma_start(out=out_d[:, :], in_=out_sb)

    # The no-accum custom-DVE instructions may legally run the 2X_1PORT
    # table program (byte-36[7:6] of the encoding; must precede compile).
    # Fused (accum_out) instructions stay in regular mode — the stream
    # accumulator does not survive fast modes on this silicon.
    for i in noacc_insts:
        i.ins.perf_max = 1

    nc.compile()
    return nc


def _get_nc():
    if "nc" not in _CACHE:
        _CACHE["nc"] = _build()
    return _CACHE["nc"]


def _consts():
    if "consts" not in _CACHE:
        _CACHE["consts"] = {
            "ident32": np.eye(P, dtype=np.float32),
            "identbn": (-np.eye(P)).astype(ml_dtypes.bfloat16),
            "ones32": np.ones((P, 1), dtype=np.float32),
        }
    return _CACHE["consts"]


def kernel(relation_logits: np.ndarray, entity_masks: np.ndarray) -> np.ndarray:
    from concourse.bass_utils import run_bass_kernel_spmd

    B, n, _, R = relation_logits.shape
    assert (n, B * R) == (N, 8)
    x = np.ascontiguousarray(
        np.transpose(np.asarray(relation_logits, dtype=np.float32), (0, 3, 1, 2))
    ).reshape(B * R, N, N)
    m = np.asarray(entity_masks) > 0
    for b in range(B):
        if not m[b].all():
            keep = np.outer(m[b], m[b])
            x[b * R:(b + 1) * R][:, ~keep] = -30.0

    consts = _consts()
    in_maps = [{"x": x[i], **consts} for i in range(8)]
    res = run_bass_kernel_spmd(_get_nc(), in_maps, list(range(8)))
    total = float(sum(float(r["out"][0, 0]) for r in res.results))
    return np.float32(WEIGHT * total / (R * B))
